# revision 13
# baseline (speedup 1.0000x reference)
"""Trainium2 Bass kernel for nn_Attention_72670846649042.

GRU encoder + greedy attention decoder, B=512,L=25,H=1024,D=256,T=128,E=300.
Sharding: data-parallel over batch, 64 rows/core on 8 cores, no collectives.
Compute dtype bf16 matmuls / fp32 state.

Transposed-orientation design (v2): all recurrent matmuls keep the state in
(feature-partition, batch-free) layout with the weights as the stationary
lhsT operand, so the full 128-wide PE array is used (the batch shard is only
64) and no per-step transposes are needed:
  gatesT(3H,B) = W(3H,H) @ h(H,B)   via lhsT=W.T tiles, rhs=h  (N=64/matmul)
Gate biases enter either as activation bias columns, or via one "selector"
matmul (K=8, lhsT=bias rows, rhs=0/1 selector) that writes bias[p,j] into
psum window j. GRU elementwise runs on (128, 8*64) slabs (full partitions).
Attention 'applied' uses the block-diag trick with enc_out as lhsT so the
result lands directly in (H-part, B) layout.
The only activation tables used inside the loops are sigmoid/tanh/relu/
identity (one set): softmax exp is computed as exp(x)=(1+tanh(x/2))/(1-
tanh(x/2)), and the final log_softmax runs in a batched epilogue (exp+ln).
"""
import os
import numpy as np
import ml_dtypes

B, L, V, E, H, D, T = 512, 25, 50000, 300, 1024, 256, 128
NC = 8
BL = B // NC          # 64 local batch
G3 = 3 * H            # 3072
KH = H // 128         # 8 hidden ktiles
NT = L * BL           # 1600 token columns
MG = G3 // 128        # 24 gate row tiles
MAXN1, MAXN2, BN_EPS = 10.0, 1.0, 1e-5
BF16 = ml_dtypes.bfloat16

LINEARIZE = False


def build_nc():
    import concourse.bass as bass
    import concourse.tile as tile
    from concourse import bacc, mybir
    from contextlib import ExitStack

    dt = mybir.dt
    AF = mybir.ActivationFunctionType
    ALU = mybir.AluOpType
    AX = mybir.AxisListType

    nc = bacc.Bacc("TRN2", target_bir_lowering=False, debug=False)

    # ---- dram parameters (per-core shards / replicated weights) ----
    xT_d = nc.declare_dram_parameter("xT", [E, NT], dt.float32, isOutput=False)
    encWihT_d = nc.declare_dram_parameter("encWihT", [E, G3], dt.bfloat16, isOutput=False)
    encWhhT_d = nc.declare_dram_parameter("encWhhT", [H, G3], dt.bfloat16, isOutput=False)
    decWihT_d = nc.declare_dram_parameter("decWihT", [H, G3], dt.bfloat16, isOutput=False)
    decWhhT_d = nc.declare_dram_parameter("decWhhT", [H, G3], dt.bfloat16, isOutput=False)
    combWT_d = nc.declare_dram_parameter("combWT", [D + H, H], dt.bfloat16, isOutput=False)
    outWTs_d = nc.declare_dram_parameter("outWTs", [H, T], dt.bfloat16, isOutput=False)
    attnWT_d = nc.declare_dram_parameter("attnWT", [D + H, L], dt.bfloat16, isOutput=False)
    embm_d = nc.declare_dram_parameter("embm", [128, D], dt.float32, isOutput=False)
    sosr_d = nc.declare_dram_parameter("sosr", [BL, D], dt.float32, isOutput=False)
    # bias columns / selector-bias rows
    egibT_d = nc.declare_dram_parameter("egibT", [128, MG], dt.float32, isOutput=False)
    sel_d = nc.declare_dram_parameter("sel", [KH, KH * BL], dt.bfloat16, isOutput=False)
    selebhn_d = nc.declare_dram_parameter("selebhn", [KH, 128], dt.bfloat16, isOutput=False)
    selr_d = nc.declare_dram_parameter("selr", [KH, 128], dt.bfloat16, isOutput=False)
    selz_d = nc.declare_dram_parameter("selz", [KH, 128], dt.bfloat16, isOutput=False)
    selgn_d = nc.declare_dram_parameter("selgn", [KH, 128], dt.bfloat16, isOutput=False)
    selin_d = nc.declare_dram_parameter("selin", [KH, 128], dt.bfloat16, isOutput=False)
    selcomb_d = nc.declare_dram_parameter("selcomb", [KH, 128], dt.bfloat16, isOutput=False)
    attnb_d = nc.declare_dram_parameter("attnb", [1, L], dt.bfloat16, isOutput=False)
    lgb_d = nc.declare_dram_parameter("lgb", [1, T], dt.bfloat16, isOutput=False)
    istk_d = nc.declare_dram_parameter("istk", [128, BL], dt.bfloat16, isOutput=False)
    ident_d = nc.declare_dram_parameter("ident", [128, 128], dt.bfloat16, isOutput=False)
    out_d = nc.declare_dram_parameter("out", [BL * L, T], dt.float32, isOutput=True)

    giT_dram = nc.dram_tensor("giT_bounce", [G3, NT], dt.float32, kind="Internal")

    with tile.TileContext(nc, linearize=LINEARIZE) as tc, ExitStack() as ctx:
        # ---------- persistent pool ----------
        shared = ctx.enter_context(tc.tile_pool(name="shared", bufs=1))

        enc_out = shared.tile([128, 13, H], dt.bfloat16, tag="enc_out")
        hT = shared.tile([128, KH * BL], dt.bfloat16, tag="hT")
        hF = shared.tile([128, KH * BL], dt.float32, tag="hF")
        embT = shared.tile([128, 2 * BL], dt.bfloat16, tag="embT")
        emb_bf = shared.tile([128, D], dt.bfloat16, tag="emb_bf")
        Istk = shared.tile([128, BL], dt.bfloat16, tag="Istk")
        ident = shared.tile([128, 128], dt.bfloat16, tag="ident")
        ones_sb = shared.tile([1, 128], dt.bfloat16, tag="ones_sb")
        selmat = shared.tile([KH, KH * BL], dt.bfloat16, tag="selmat")
        attnb_r = shared.tile([1, L], dt.bfloat16, tag="attnb_r")
        lgb_r = shared.tile([1, T], dt.bfloat16, tag="lgb_r")

        nc.sync.dma_start(Istk[:], istk_d.ap())
        nc.sync.dma_start(ident[:], ident_d.ap())
        nc.sync.dma_start(selmat[:], sel_d.ap())
        nc.sync.dma_start(attnb_r[:], attnb_d.ap())
        nc.sync.dma_start(lgb_r[:], lgb_d.ap())
        nc.vector.memset(ones_sb[:], 1.0)
        nc.vector.memset(hF[:], 0.0)
        nc.vector.memset(hT[:], 0.0)
        nc.vector.memset(enc_out[BL:128, 12, :], 0.0)

        # ---- pre: dec_emb renorm rows 0..127 -> emb_bf; SOS -> embT ----
        with tc.tile_pool(name="pre", bufs=1) as pre:
            embm = pre.tile([128, D], dt.float32, tag="embm")
            nc.sync.dma_start(embm[:], embm_d.ap())
            sq = pre.tile([128, D], dt.float32, tag="sq")
            nc.vector.tensor_tensor(sq[:], embm[:], embm[:], op=ALU.mult)
            ssum = pre.tile([128, 1], dt.float32, tag="ssum")
            nc.vector.tensor_reduce(ssum[:], sq[:], axis=AX.X, op=ALU.add)
            nrm = pre.tile([128, 1], dt.float32, tag="nrm")
            nc.scalar.activation(nrm[:], ssum[:], AF.Sqrt)
            nc.vector.tensor_scalar(nrm[:], nrm[:], 1e-7, None, op0=ALU.add)
            rcp = pre.tile([128, 1], dt.float32, tag="rcp")
            nc.vector.reciprocal(rcp[:], nrm[:])
            scl = pre.tile([128, 1], dt.float32, tag="scl")
            nc.vector.tensor_scalar(scl[:], rcp[:], MAXN2, 1.0, op0=ALU.mult, op1=ALU.min)
            nc.vector.tensor_scalar(emb_bf[:], embm[:], scl[:], None, op0=ALU.mult)

            sos = pre.tile([BL, D], dt.float32, tag="sos")
            nc.sync.dma_start(sos[:], sosr_d.ap())
            sq2 = pre.tile([BL, D], dt.float32, tag="sq2")
            nc.vector.tensor_tensor(sq2[:], sos[:], sos[:], op=ALU.mult)
            ssum2 = pre.tile([BL, 1], dt.float32, tag="ssum2")
            nc.vector.tensor_reduce(ssum2[:], sq2[:], axis=AX.X, op=ALU.add)
            nrm2 = pre.tile([BL, 1], dt.float32, tag="nrm2")
            nc.scalar.activation(nrm2[:], ssum2[:], AF.Sqrt)
            nc.vector.tensor_scalar(nrm2[:], nrm2[:], 1e-7, None, op0=ALU.add)
            rcp2 = pre.tile([BL, 1], dt.float32, tag="rcp2")
            nc.vector.reciprocal(rcp2[:], nrm2[:])
            scl2 = pre.tile([BL, 1], dt.float32, tag="scl2")
            nc.vector.tensor_scalar(scl2[:], rcp2[:], MAXN2, 1.0, op0=ALU.mult, op1=ALU.min)
            sos_bf = pre.tile([BL, D], dt.bfloat16, tag="sos_bf")
            nc.vector.tensor_scalar(sos_bf[:], sos[:], scl2[:], None, op0=ALU.mult)
            for k in range(2):
                nc.sync.dma_start_transpose(embT[:, k * BL:(k + 1) * BL],
                                            sos_bf[:, k * 128:(k + 1) * 128])

        # =======================================================
        # Phase 1: giT = encWih @ renorm(x).T  -> giT_dram (fp32)
        # (egib = bih + bhh_{r,z}-folded biases added via act bias col)
        # =======================================================
        CH = ((0, 512), (512, 1024), (1024, 1536), (1536, 1600))
        kr = (128, 128, 44)
        with tc.tile_pool(name="p1", bufs=1) as p1, \
             tc.tile_pool(name="p1g", bufs=3) as p1g, \
             tc.tile_pool(name="p1ps", bufs=4, space="PSUM") as p1ps, \
             tc.tile_pool(name="p1nps", bufs=2, space="PSUM") as p1nps:
            encWihT = p1.tile([128, 3, G3], dt.bfloat16, tag="encWihT")
            nc.sync.dma_start(encWihT[:, 0, :], encWihT_d.ap()[0:128, :])
            nc.sync.dma_start(encWihT[:, 1, :], encWihT_d.ap()[128:256, :])
            nc.sync.dma_start(encWihT[0:44, 2, :], encWihT_d.ap()[256:300, :])
            egibT = p1.tile([128, MG], dt.float32, tag="egibT")
            nc.sync.dma_start(egibT[:], egibT_d.ap())
            onesk = p1.tile([128, 1], dt.bfloat16, tag="onesk")
            nc.vector.memset(onesk[:], 1.0)

            xt = p1.tile([128, 3, NT], dt.float32, tag="xt")
            nc.sync.dma_start(xt[:, 0, :], xT_d.ap()[0:128, :])
            nc.sync.dma_start(xt[:, 1, :], xT_d.ap()[128:256, :])
            nc.sync.dma_start(xt[0:44, 2, :], xT_d.ap()[256:300, :])
            # col squared-norms -> scale row = 1/sqrt(max(1, s/100))
            xsq = p1.tile([128, 3, NT], dt.bfloat16, tag="xsq")
            for k in range(3):
                nc.vector.tensor_tensor(xsq[0:kr[k], k, :], xt[0:kr[k], k, :],
                                        xt[0:kr[k], k, :], op=ALU.mult)
            sclrow = p1.tile([1, NT], dt.float32, tag="sclrow")
            for c, (c0, c1) in enumerate(CH):
                npsc = p1nps.tile([1, 512], dt.float32, tag="nps")
                for k in range(3):
                    nc.tensor.matmul(npsc[0:1, 0:c1 - c0], onesk[0:kr[k], :],
                                     xsq[0:kr[k], k, c0:c1], start=(k == 0),
                                     stop=(k == 2))
                nc.vector.tensor_scalar(sclrow[:, c0:c1], npsc[0:1, 0:c1 - c0],
                                        0.01, 1.0, op0=ALU.mult, op1=ALU.max)
            nc.scalar.activation(sclrow[:], sclrow[:], AF.Sqrt)
            nc.vector.reciprocal(sclrow[:], sclrow[:])
            sclbf = p1.tile([1, NT], dt.bfloat16, tag="sclbf")
            nc.vector.tensor_copy(sclbf[:], sclrow[:])
            # broadcast scale over partitions, then xbf = x * scale (bf16)
            sclb = p1.tile([128, NT], dt.bfloat16, tag="sclb")
            for c, (c0, c1) in enumerate(CH):
                bps = p1nps.tile([128, 512], dt.float32, tag="bps")
                nc.tensor.matmul(bps[:, 0:c1 - c0], ones_sb[0:1, :],
                                 sclbf[:, c0:c1], start=True, stop=True)
                nc.scalar.copy(sclb[:, c0:c1], bps[:, 0:c1 - c0])
            xbf = p1.tile([128, 3, NT], dt.bfloat16, tag="xbf")
            for k in range(3):
                nc.vector.tensor_tensor(xbf[0:kr[k], k, :], xt[0:kr[k], k, :],
                                        sclb[0:kr[k], :], op=ALU.mult)
            # main: giT rows mt*128.. = encWih rows @ xbf  (+ egib bias col)
            for mt in range(MG):
                gsb = p1g.tile([128, NT], dt.float32, tag="gsb")
                for c, (c0, c1) in enumerate(CH):
                    gps = p1ps.tile([128, 512], dt.float32, tag="gps")
                    for k in range(3):
                        nc.tensor.matmul(gps[:, 0:c1 - c0],
                                         encWihT[0:kr[k], k, mt * 128:(mt + 1) * 128],
                                         xbf[0:kr[k], k, c0:c1],
                                         start=(k == 0), stop=(k == 2))
                    if c % 2 == 0:
                        nc.scalar.activation(gsb[:, c0:c1], gps[:, 0:c1 - c0],
                                             AF.Identity, bias=egibT[:, mt:mt + 1])
                    else:
                        nc.vector.tensor_scalar(gsb[:, c0:c1], gps[:, 0:c1 - c0],
                                                egibT[:, mt:mt + 1], None, op0=ALU.add)
                nc.sync.dma_start(giT_dram.ap()[mt * 128:(mt + 1) * 128, :], gsb[:])

        # =======================================================
        # Phase 2: encoder GRU scan (25 steps), transposed layout
        # r slab windows j: gate rows j*128..; z: H+j*128; gn: 2H+j*128
        # =======================================================
        # dec weights preloaded here so their DMA overlaps the enc scan
        decw = ctx.enter_context(tc.tile_pool(name="decw", bufs=1))
        decWhhT = decw.tile([128, KH, G3], dt.bfloat16, tag="decWhhT")
        nc.gpsimd.dma_start(decWhhT[:], decWhhT_d.ap().rearrange("(k p) n -> p k n", p=128))
        combWT = decw.tile([128, 10, H], dt.bfloat16, tag="combWT")
        nc.gpsimd.dma_start(combWT[:], combWT_d.ap().rearrange("(k p) n -> p k n", p=128))
        outWTs = decw.tile([128, KH, T], dt.bfloat16, tag="outWTs")
        nc.gpsimd.dma_start(outWTs[:], outWTs_d.ap().rearrange("(k p) n -> p k n", p=128))
        attnWT = decw.tile([128, 10, L], dt.bfloat16, tag="attnWT")
        nc.gpsimd.dma_start(attnWT[:], attnWT_d.ap().rearrange("(k p) n -> p k n", p=128))
        selr = decw.tile([KH, 128], dt.bfloat16, tag="selr")
        nc.gpsimd.dma_start(selr[:], selr_d.ap())
        selz = decw.tile([KH, 128], dt.bfloat16, tag="selz")
        nc.gpsimd.dma_start(selz[:], selz_d.ap())
        selgn = decw.tile([KH, 128], dt.bfloat16, tag="selgn")
        nc.gpsimd.dma_start(selgn[:], selgn_d.ap())
        selin = decw.tile([KH, 128], dt.bfloat16, tag="selin")
        nc.gpsimd.dma_start(selin[:], selin_d.ap())
        selcomb = decw.tile([KH, 128], dt.bfloat16, tag="selcomb")
        nc.gpsimd.dma_start(selcomb[:], selcomb_d.ap())

        def win(sl, j):
            return sl[:, j * BL:(j + 1) * BL]

        with tc.tile_pool(name="encw", bufs=1) as encw, \
             tc.tile_pool(name="egi", bufs=3) as egi, \
             tc.tile_pool(name="eps", bufs=1, space="PSUM") as eps, \
             tc.tile_pool(name="etps", bufs=2, space="PSUM") as etps, \
             tc.tile_pool(name="ework", bufs=1) as ework:
            encWhhT = encw.tile([128, KH, G3], dt.bfloat16, tag="encWhhT")
            nc.sync.dma_start(encWhhT[:], encWhhT_d.ap().rearrange("(k p) n -> p k n", p=128))
            selebhn = encw.tile([KH, 128], dt.bfloat16, tag="selebhn")
            nc.sync.dma_start(selebhn[:], selebhn_d.ap())
            for t in range(L):
                gi = egi.tile([128, MG, BL], dt.float32, tag="gi")
                nc.sync.dma_start(
                    gi[:], giT_dram.ap()[:, t * BL:(t + 1) * BL]
                    .rearrange("(m p) b -> p m b", p=128))
                ps_r = eps.tile([128, KH * BL], dt.float32, tag="ps_r")
                ps_z = eps.tile([128, KH * BL], dt.float32, tag="ps_z")
                ps_gn = eps.tile([128, KH * BL], dt.float32, tag="ps_gn")
                # gn gets bhh_n via selector; r/z biases already in gi
                nc.tensor.matmul(ps_gn[:], selebhn[:], selmat[:], start=True, stop=False)
                for g, ps in ((0, ps_r), (2, ps_gn), (1, ps_z)):
                    for j in range(KH):
                        nco = g * H + j * 128
                        for k in range(KH):
                            nc.tensor.matmul(win(ps, j), encWhhT[:, k, nco:nco + 128],
                                             win(hT, k),
                                             start=(g != 2 and j == 0 and k == 0),
                                             stop=(j == KH - 1 and k == KH - 1))
                # gates chain (slab-wide)
                t_r = ework.tile([128, KH * BL], dt.float32, tag="t_r")
                nc.vector.tensor_tensor(t_r[:], ps_r[:], gi[:, 0:KH, :], op=ALU.add)
                r_s = ework.tile([128, KH * BL], dt.float32, tag="r_s")
                nc.scalar.activation(r_s[:], t_r[:], AF.Sigmoid)
                m1 = ework.tile([128, KH * BL], dt.float32, tag="m1")
                nc.vector.tensor_tensor(m1[:], ps_gn[:], r_s[:], op=ALU.mult)
                nc.vector.tensor_tensor(m1[:], m1[:], gi[:, 2 * KH:3 * KH, :], op=ALU.add)
                n_s = ework.tile([128, KH * BL], dt.float32, tag="n_s")
                nc.scalar.activation(n_s[:], m1[:], AF.Tanh)
                t_z = ework.tile([128, KH * BL], dt.float32, tag="t_z")
                nc.vector.tensor_tensor(t_z[:], ps_z[:], gi[:, KH:2 * KH, :], op=ALU.add)
                z_s = ework.tile([128, KH * BL], dt.float32, tag="z_s")
                nc.scalar.activation(z_s[:], t_z[:], AF.Sigmoid)
                t4 = ework.tile([128, KH * BL], dt.float32, tag="t4")
                nc.vector.tensor_tensor(t4[:], hF[:], n_s[:], op=ALU.subtract)
                nc.vector.tensor_tensor(t4[:], t4[:], z_s[:], op=ALU.mult)
                nc.vector.tensor_tensor(hF[:], n_s[:], t4[:], op=ALU.add)
                nc.scalar.copy(hT[:], hF[:])
                # enc_out (tok-part layout) via PE transposes of hT windows
                po = (t % 2) * BL
                for j in range(KH):
                    tp = etps.tile([BL, 128], dt.bfloat16, tag="tp")
                    nc.tensor.transpose(tp[:], win(hT, j), ident[:])
                    nc.scalar.copy(enc_out[po:po + BL, t // 2, j * 128:(j + 1) * 128],
                                   tp[:])

        # =======================================================
        # Phase 3: decoder (25 steps) + log_softmax epilogue
        # =======================================================
        with tc.tile_pool(name="decw2", bufs=1) as decw2, \
             tc.tile_pool(name="dec", bufs=2) as decp, \
             tc.tile_pool(name="dgps", bufs=1, space="PSUM") as dgps, \
             tc.tile_pool(name="dsps", bufs=1, space="PSUM") as dsps, \
             tc.tile_pool(name="dwork", bufs=1) as dwork, \
             tc.tile_pool(name="lgp", bufs=1) as lgp:
            decWihT = decw2.tile([128, KH, G3], dt.bfloat16, tag="decWihT")
            nc.gpsimd.dma_start(decWihT[:], decWihT_d.ap().rearrange("(k p) n -> p k n", p=128))
            lgstore = lgp.tile([BL, L, T], dt.float32, tag="lgstore")
            for t in range(L):
                # ---- attention scores (64b, 25l) ----
                scps = dsps.tile([BL, 512], dt.float32, tag="ps")
                for k in range(10):
                    lhs = embT[:, k * BL:(k + 1) * BL] if k < 2 else win(hT, k - 2)
                    nc.tensor.matmul(scps[:, 0:L], lhs, attnWT[:, k, :],
                                     start=(k == 0), stop=False)
                nc.tensor.matmul(scps[:, 0:L], ones_sb[0:1, 0:BL], attnb_r[:],
                                 start=False, stop=True)
                # softmax via exp(x) = (1+tanh(x/2))/(1-tanh(x/2)) (no table switch)
                t_aw = decp.tile([BL, L], dt.float32, tag="t_aw")
                nc.scalar.activation(t_aw[:], scps[:, 0:L], AF.Tanh, scale=0.5)
                u_aw = decp.tile([BL, L], dt.float32, tag="u_aw")
                nc.vector.tensor_scalar(u_aw[:], t_aw[:], -1.0, 1.0, op0=ALU.mult,
                                        op1=ALU.add)
                nc.vector.reciprocal(u_aw[:], u_aw[:])
                w_aw = decp.tile([BL, L], dt.float32, tag="w_aw")
                nc.vector.tensor_scalar(w_aw[:], t_aw[:], 1.0, None, op0=ALU.add)
                aw = decp.tile([BL, L], dt.float32, tag="aw")
                nc.vector.tensor_tensor(aw[:], w_aw[:], u_aw[:], op=ALU.mult)
                sume = decp.tile([BL, 1], dt.float32, tag="sume")
                nc.vector.tensor_reduce(sume[:], aw[:], axis=AX.X, op=ALU.add)
                rs = decp.tile([BL, 1], dt.float32, tag="rs")
                nc.vector.reciprocal(rs[:], sume[:])
                # awsh (128, L): top=aw, bottom=aw shifted left 1 (pad 0)
                awsh = decp.tile([128, L], dt.float32, tag="awsh")
                nc.vector.memset(awsh[BL:128, L - 1:L], 0.0)
                nc.vector.tensor_copy(awsh[0:BL, :], aw[:])
                nc.vector.tensor_copy(awsh[BL:128, 0:L - 1], aw[:, 1:L])
                rs2 = decp.tile([128, 1], dt.float32, tag="rs2")
                nc.vector.tensor_copy(rs2[0:BL, :], rs[:])
                nc.vector.tensor_copy(rs2[BL:128, :], rs[:])
                # ---- GRU psum slabs: selector bias matmuls first ----
                ps_r = dgps.tile([128, KH * BL], dt.float32, tag="ps_r")
                ps_z = dgps.tile([128, KH * BL], dt.float32, tag="ps_z")
                ps_gn = dgps.tile([128, KH * BL], dt.float32, tag="ps_gn")
                ps_in = dgps.tile([128, KH * BL], dt.float32, tag="ps_in")
                nc.tensor.matmul(ps_r[:], selr[:], selmat[:], start=True, stop=False)
                nc.tensor.matmul(ps_z[:], selz[:], selmat[:], start=True, stop=False)
                nc.tensor.matmul(ps_gn[:], selgn[:], selmat[:], start=True, stop=False)
                nc.tensor.matmul(ps_in[:], selin[:], selmat[:], start=True, stop=False)
                # Whh streams (need only hT): r, gn, z
                for g, ps in ((0, ps_r), (2, ps_gn), (1, ps_z)):
                    for j in range(KH):
                        nco = g * H + j * 128
                        for k in range(KH):
                            nc.tensor.matmul(win(ps, j), decWhhT[:, k, nco:nco + 128],
                                             win(hT, k), start=False,
                                             stop=(g == 2 and j == KH - 1
                                                   and k == KH - 1))
                # ---- applied: dgs block-diag rhs, enc_out as lhsT ----
                dgs = decp.tile([128, 13 * BL], dt.bfloat16, tag="dgs")
                for p in range(13):
                    nc.vector.tensor_scalar(dgs[:, p * BL:(p + 1) * BL], Istk[:],
                                            awsh[:, 2 * p:2 * p + 1], rs2[:],
                                            op0=ALU.mult, op1=ALU.mult)
                aps = dsps.tile([128, KH * BL], dt.float32, tag="ps2")
                for j in range(KH):
                    for p in range(13):
                        nc.tensor.matmul(win(aps, j), enc_out[:, p, j * 128:(j + 1) * 128],
                                         dgs[:, p * BL:(p + 1) * BL],
                                         start=(j == 0 and p == 0),
                                         stop=(j == KH - 1 and p == 12))
                apT = decp.tile([128, KH * BL], dt.bfloat16, tag="apT")
                nc.scalar.copy(apT[:], aps[:])
                # ---- comb + bn2 + relu -> oT (H-part, b) ----
                cps = dsps.tile([128, KH * BL], dt.float32, tag="ps3")
                nc.tensor.matmul(cps[:], selcomb[:], selmat[:], start=True, stop=False)
                for j in range(KH):
                    for k in range(10):
                        rhs = embT[:, k * BL:(k + 1) * BL] if k < 2 else win(apT, k - 2)
                        nc.tensor.matmul(win(cps, j), combWT[:, k, j * 128:(j + 1) * 128],
                                         rhs, start=False,
                                         stop=(k == 9 and j == KH - 1))
                oT = decp.tile([128, KH * BL], dt.bfloat16, tag="oT")
                nc.scalar.activation(oT[:], cps[:], AF.Relu, scale=S2_SCALE)
                # ---- Wih streams (need oT): r, in, z ----
                for g, ps, nbase in ((0, ps_r, 0), (2, ps_in, 2 * H), (1, ps_z, H)):
                    for j in range(KH):
                        nco = nbase + j * 128
                        for k in range(KH):
                            nc.tensor.matmul(win(ps, j), decWihT[:, k, nco:nco + 128],
                                             win(oT, k), start=False,
                                             stop=(j == KH - 1 and k == KH - 1))
                # ---- gates chain ----
                r_s = dwork.tile([128, KH * BL], dt.float32, tag="r_s")
                nc.scalar.activation(r_s[:], ps_r[:], AF.Sigmoid)
                m1 = dwork.tile([128, KH * BL], dt.float32, tag="m1")
                nc.vector.tensor_tensor(m1[:], ps_gn[:], r_s[:], op=ALU.mult)
                nc.vector.tensor_tensor(m1[:], m1[:], ps_in[:], op=ALU.add)
                n_s = dwork.tile([128, KH * BL], dt.float32, tag="n_s")
                nc.scalar.activation(n_s[:], m1[:], AF.Tanh)
                z_s = dwork.tile([128, KH * BL], dt.float32, tag="z_s")
                nc.scalar.activation(z_s[:], ps_z[:], AF.Sigmoid)
                t4 = dwork.tile([128, KH * BL], dt.float32, tag="t4")
                nc.vector.tensor_tensor(t4[:], hF[:], n_s[:], op=ALU.subtract)
                nc.vector.tensor_tensor(t4[:], t4[:], z_s[:], op=ALU.mult)
                nc.vector.tensor_tensor(hF[:], n_s[:], t4[:], op=ALU.add)
                nc.scalar.copy(hT[:], hF[:])
                # ---- logits (64b, T) = hT.T @ outWTs + lgb (bn1 folded) ----
                lps = dsps.tile([BL, 512], dt.float32, tag="ps")
                for k in range(KH):
                    nc.tensor.matmul(lps[:, 0:T], win(hT, k), outWTs[:, k, :],
                                     start=(k == 0), stop=False)
                nc.tensor.matmul(lps[:, 0:T], ones_sb[0:1, 0:BL], lgb_r[:],
                                 start=False, stop=True)
                nc.vector.tensor_copy(lgstore[:, t, :], lps[:, 0:T])
                # ---- argmax -> onehot -> next embT (skip at last step) ----
                if t < L - 1:
                    mx2 = decp.tile([BL, 1], dt.float32, tag="mx2")
                    nc.vector.tensor_reduce(mx2[:], lps[:, 0:T], axis=AX.X, op=ALU.max)
                    oh = decp.tile([BL, T], dt.bfloat16, tag="oh")
                    nc.vector.tensor_scalar(oh[:], lps[:, 0:T], mx2[:], None,
                                            op0=ALU.is_equal)
                    ohps = dsps.tile([T, BL], dt.bfloat16, tag="ps4")
                    nc.tensor.transpose(ohps[:], oh[:], ident[0:BL, 0:BL])
                    ohT = decp.tile([T, BL], dt.bfloat16, tag="ohT")
                    nc.scalar.copy(ohT[:], ohps[:])
                    eps2 = dsps.tile([128, KH * BL], dt.float32, tag="ps3")
                    for k in range(2):
                        nc.tensor.matmul(eps2[:, k * BL:(k + 1) * BL],
                                         emb_bf[:, k * 128:(k + 1) * 128],
                                         ohT[:], start=(k == 0), stop=(k == 1))
                    nc.scalar.copy(embT[:], eps2[:, 0:2 * BL])
            # ---- epilogue: log_softmax over stored logits ----
            with tc.tile_pool(name="epi", bufs=3) as epi:
                for t in range(L):
                    mx = epi.tile([BL, 1], dt.float32, tag="mx")
                    nc.vector.tensor_reduce(mx[:], lgstore[:, t, :], axis=AX.X,
                                            op=ALU.max)
                    nmx = epi.tile([BL, 1], dt.float32, tag="nmx")
                    nc.vector.tensor_scalar(nmx[:], mx[:], -1.0, None, op0=ALU.mult)
                    ex = epi.tile([BL, T], dt.float32, tag="ex")
                    se = epi.tile([BL, 1], dt.float32, tag="se")
                    nc.scalar.activation(ex[:], lgstore[:, t, :], AF.Exp, bias=nmx[:],
                                         accum_out=se[:])
                    lse = epi.tile([BL, 1], dt.float32, tag="lse")
                    nc.scalar.activation(lse[:], se[:], AF.Ln)
                    nc.vector.tensor_tensor(lse[:], lse[:], mx[:], op=ALU.add)
                    lout = epi.tile([BL, T], dt.float32, tag="lout")
                    nc.vector.tensor_scalar(lout[:], lgstore[:, t, :], lse[:], None,
                                            op0=ALU.subtract)
                    nc.sync.dma_start(
                        out_d.ap().rearrange("(b l) c -> b l c", l=L)[:, t, :], lout[:])
    nc.finalize()
    return nc


S2_SCALE = 1.0  # patched at build time (bn2 scale); module-level for closure use


def prepare_in_maps(inputs):
    """Host-side preprocessing: returns per-core input maps (sets S2_SCALE)."""
    global S2_SCALE
    tokens = np.asarray(inputs["tokens"])
    w2v = np.asarray(inputs["w2v"], np.float32)
    bn1 = np.asarray(inputs["bn1"], np.float32)
    bn2 = np.asarray(inputs["bn2"], np.float32)
    s1 = float(bn1[0] / np.sqrt(bn1[3] + BN_EPS))
    t1 = float(bn1[1] - bn1[2] * s1)
    s2 = float(bn2[0] / np.sqrt(bn2[3] + BN_EPS))
    t2 = float(bn2[1] - bn2[2] * s2)
    S2_SCALE = s2

    f32 = lambda k: np.asarray(inputs[k], np.float32)
    bft = lambda a: np.ascontiguousarray(np.asarray(a, np.float32).T).astype(BF16)
    enc_bih, enc_bhh = f32("enc_bih"), f32("enc_bhh")
    dec_bih, dec_bhh = f32("dec_bih"), f32("dec_bhh")
    # enc: r/z biases (bih+bhh) and bih_n folded into giT; bhh_n via selector
    egib = np.concatenate([enc_bih[:H] + enc_bhh[:H], enc_bih[H:2 * H] + enc_bhh[H:2 * H],
                           enc_bih[2 * H:]])
    egibT = np.ascontiguousarray(egib.reshape(MG, 128).T)          # (128, 24)
    out_W = f32("out_W")
    outWTs = np.ascontiguousarray((s1 * out_W).T).astype(BF16)
    lgb = (f32("out_b") + t1 * out_W.sum(axis=1))[None, :]
    combb = (f32("comb_b") + t2 / s2)

    sel = np.zeros((KH, KH * BL), np.float32)
    for j in range(KH):
        sel[j, j * BL:(j + 1) * BL] = 1.0

    istk = np.zeros((128, BL), np.float32)
    istk[np.arange(128), np.arange(128) % BL] = 1.0

    common = {
        "encWihT": bft(inputs["enc_Wih"]), "encWhhT": bft(inputs["enc_Whh"]),
        "decWihT": bft(inputs["dec_Wih"]), "decWhhT": bft(inputs["dec_Whh"]),
        "combWT": bft(inputs["comb_W"]), "outWTs": outWTs,
        "attnWT": bft(inputs["attn_W"]),
        "embm": np.asarray(inputs["dec_emb"][:128], np.float32),
        "sosr": np.ascontiguousarray(
            np.broadcast_to(np.asarray(inputs["dec_emb"][T], np.float32), (BL, D))),
        "egibT": egibT,
        "sel": sel.astype(BF16),
        "selebhn": np.ascontiguousarray(enc_bhh[2 * H:].reshape(KH, 128)).astype(BF16),
        "selr": np.ascontiguousarray(
            (dec_bih[:H] + dec_bhh[:H]).reshape(KH, 128)).astype(BF16),
        "selz": np.ascontiguousarray(
            (dec_bih[H:2 * H] + dec_bhh[H:2 * H]).reshape(KH, 128)).astype(BF16),
        "selgn": np.ascontiguousarray(dec_bhh[2 * H:].reshape(KH, 128)).astype(BF16),
        "selin": np.ascontiguousarray(dec_bih[2 * H:].reshape(KH, 128)).astype(BF16),
        "selcomb": np.ascontiguousarray(combb.reshape(KH, 128)).astype(BF16),
        "attnb": np.ascontiguousarray(f32("attn_b")[None, :]).astype(BF16),
        "lgb": np.ascontiguousarray(lgb).astype(BF16),
        "istk": istk.astype(BF16),
        "ident": np.eye(128, dtype=np.float32).astype(BF16),
    }
    in_maps = []
    for c in range(NC):
        tok = tokens[c * BL:(c + 1) * BL].astype(np.int64)        # (64,25)
        xg = w2v[tok]                                             # (64,25,300)
        # column index = l*64 + b
        xT = np.ascontiguousarray(xg.transpose(2, 1, 0).reshape(E, NT))
        m = dict(common)
        m["xT"] = xT
        in_maps.append(m)
    return in_maps


def kernel(**inputs):
    import concourse.bass_utils as bass_utils

    in_maps = prepare_in_maps(inputs)
    nc = build_nc()
    res = bass_utils.run_bass_kernel_spmd(nc, in_maps, core_ids=list(range(NC)))
    out = np.concatenate([res.results[c]["out"] for c in range(NC)], axis=0)
    return out.astype(np.float32)


if __name__ == "__main__":
    pass
